# revision 14
# baseline (speedup 1.0000x reference)
"""DeepSeekV2-style MLA prefill attention on 8 Trainium2 NeuronCores.

Tensor-parallel over heads: each core owns 2 of the 16 q heads (q_nope only),
replicates the single latent kv head, computes its heads' causal attention
and a partial o-proj; the host sums the 8 partial outputs.

v2 design notes (driven by the p-state clock model: PE runs 0.65/1.2 GHz
after any idle gap and only reaches 2.4 GHz after 3us of continuous work, so
the whole kernel is scheduled to keep the tensor engine hot):

  - warmup transposes at t=0 start the clock ramp during the first DMA wait
  - proj phase is split into two 4-bank PSUM half-passes per s-tile so bank
    drains overlap the next pass's fills (no st2-boundary stall); v is
    computed directly in natural [l, d] layout (hs slice as the stationary
    operand), eliminating the 32 PE transposes of v
  - attention q-tiles are 1024 wide: one exp() activation per 2-bank PSUM
    scores tile halves the scalar engine's per-call overhead
  - the scalar engine does ONLY exp; every PSUM->SBUF drain runs on the
    Pool (gpsimd) or DVE (vector) engine
  - the scores loop is exp-gated (~1us exp vs ~430ns of matmul per l-chunk),
    so a work queue interleaves the PREVIOUS group's PV passes / normalizes
    and pending o-proj column blocks into the gaps, in program order
"""

import numpy as np
import ml_dtypes
from collections import deque
from contextlib import ExitStack

import concourse.bass as bass
import concourse.mybir as mybir
import concourse.tile as tile
from concourse import bacc
from concourse.bass_utils import run_bass_kernel_spmd
from concourse.masks import make_identity

B, S, HID = 2, 2048, 2048
H, D_NOPE, D_ROPE = 16, 128, 64
D = D_NOPE
N_CORES = 8
HPC = H // N_CORES          # heads per core
BS = B * S
SCALE = float(D_NOPE) ** -0.5

F32 = mybir.dt.float32
BF16 = mybir.dt.bfloat16

HC = HID // 128             # 16 hid chunks
ST2 = BS // 1024            # 4 wide s tiles
LCB = S // 128              # 16 l chunks per batch
QT = S // 1024              # 2 q tiles (1024 wide) per batch

_cache = {}


def _build():
    if "nc" in _cache:
        return _cache["nc"]

    nc = bacc.Bacc("TRN2", target_bir_lowering=False, debug=False,
                   num_devices=N_CORES)
    # hsT tiled: [st2, hid_chunk, 128, 1024] contiguous blocks
    hsT_d = nc.dram_tensor("hsTt", [ST2, HC, 128, 1024], BF16,
                           kind="ExternalInput").ap()
    wqT_d = nc.dram_tensor("wqT", [HID, HPC * D], BF16, kind="ExternalInput").ap()
    wkT_d = nc.dram_tensor("wkT", [HID, D], BF16, kind="ExternalInput").ap()
    wvT_d = nc.dram_tensor("wvT", [HID, D], BF16, kind="ExternalInput").ap()
    woT_d = nc.dram_tensor("woT", [HPC * D, HID], BF16, kind="ExternalInput").ap()
    # output tiled: [hid_chunk, st2, 128, 1024]
    outT_d = nc.dram_tensor("outTt", [HC, ST2, 128, 1024], BF16,
                            kind="ExternalOutput").ap()

    with ExitStack() as ctx:
        tc = ctx.enter_context(tile.TileContext(nc))
        persist = ctx.enter_context(tc.tile_pool(name="persist", bufs=1))

        wq_sb = persist.tile([128, HC, HPC * D], BF16, tag="wq_sb")
        wk_sb = persist.tile([128, HC, D], BF16, tag="wk_sb")
        wv_sb = persist.tile([128, HC, D], BF16, tag="wv_sb")
        wo_sb = persist.tile([128, HPC, HC, 128], BF16, tag="wo_sb")
        qT_sb = persist.tile([128, HPC, BS], BF16, tag="qT_sb")
        kT_sb = persist.tile([128, BS], BF16, tag="kT_sb")
        vT_sb = persist.tile([128, BS], BF16, tag="vT_sb")
        v_sb = persist.tile([128, B * LCB, D + 1], BF16, tag="v_sb")
        tri_f = persist.tile([128, 128], F32, tag="tri_f")
        tri_sb = persist.tile([128, 128], BF16, tag="tri_sb")
        ident_b = persist.tile([128, 128], BF16, tag="ident_b")
        outT_sb = persist.tile([128, HPC, BS], BF16, tag="outT_sb")

        # ---- constants ----
        wq_src = wqT_d.rearrange("(c p) m -> p c m", p=128)
        wk_src = wkT_d.rearrange("(c p) m -> p c m", p=128)
        wv_src = wvT_d.rearrange("(c p) m -> p c m", p=128)
        make_identity(nc, ident_b[:])
        # tri[x, y] = 1 where x <= y else 0 (diag-block causal mask)
        nc.gpsimd.memset(tri_f[:], 1.0)
        nc.gpsimd.affine_select(
            out=tri_f[:], in_=tri_f[:],
            compare_op=mybir.AluOpType.is_ge,
            fill=0.0, base=0,
            pattern=[[1, 128]], channel_multiplier=-1)
        nc.vector.tensor_copy(tri_sb[:], tri_f[:])
        nc.vector.memset(v_sb[:, :, D:D + 1], 1.0)

        # ---- phase 1: projections ----
        # per st2: pass A (hf=0 q/k halves + all 8 natural-layout v tiles),
        # pass B (hf=1 q/k halves) while pass A's banks drain.
        with tc.tile_pool(name="ps_proj", bufs=1, space="PSUM") as ps_proj, \
             tc.tile_pool(name="hs_pool", bufs=18) as hs_pool:
            # warmup: keep PE busy during the initial DMA wait so the clock
            # ramp starts immediately (values are discarded: the first v
            # matmul below starts a fresh accumulation group).
            pv_warm = ps_proj.tile([128, 512], F32, tag="pv", bufs=2,
                                   name="pv_warm")
            for w in range(8):
                nc.tensor.transpose(pv_warm[:, 0:128], tri_f[:], tri_f[:])

            for st2 in range(ST2):
                hsts = []
                for hcx in range(HC):
                    hst = hs_pool.tile([128, 1024], BF16, tag="hst")
                    nc.sync.dma_start(hst[:], hsT_d[st2, hcx])
                    hsts.append(hst)
                    if st2 == 0:
                        nc.sync.dma_start(wq_sb[:, hcx, :], wq_src[:, hcx, :])
                        nc.sync.dma_start(wk_sb[:, hcx, :], wk_src[:, hcx, :])
                        nc.sync.dma_start(wv_sb[:, hcx, :], wv_src[:, hcx, :])
                        if hcx == HC - 1:
                            nc.sync.dma_start(
                                wo_sb[:],
                                woT_d.rearrange("(h p) (c m) -> p h c m",
                                                p=128, m=128))
                for hf in range(2):
                    pq = [ps_proj.tile([128, 512], F32, tag=f"pq{h}", bufs=2,
                                       name=f"pq{h}_{hf}") for h in range(HPC)]
                    pk = ps_proj.tile([128, 512], F32, tag="pk", bufs=2,
                                      name=f"pk_{hf}")
                    pv = ps_proj.tile([128, 512], F32, tag="pv", bufs=2,
                                      name=f"pv_{hf}")
                    for hcx in range(HC):
                        hst = hsts[hcx]
                        hr = hst[:, hf * 512:(hf + 1) * 512]
                        first, last = hcx == 0, hcx == HC - 1
                        for h in range(HPC):
                            nc.tensor.matmul(
                                pq[h][:], wq_sb[:, hcx, h * D:(h + 1) * D],
                                hr, start=first, stop=last)
                        nc.tensor.matmul(pk[:], wk_sb[:, hcx, :], hr,
                                         start=first, stop=last)
                        nc.tensor.matmul(pv[:], wv_sb[:, hcx, :], hr,
                                         start=first, stop=last)
                    sl = slice(st2 * 1024 + hf * 512,
                               st2 * 1024 + (hf + 1) * 512)
                    for h in range(HPC):
                        if h == 0:
                            nc.scalar.copy(qT_sb[:, h, sl], pq[h][:])
                        else:
                            nc.vector.tensor_copy(qT_sb[:, h, sl], pq[h][:])
                    nc.scalar.copy(kT_sb[:, sl], pk[:])
                    nc.vector.tensor_copy(vT_sb[:, sl], pv[:])

        # ---- phase 2: attention + o-proj, one shared 8-bank PSUM pool ----
        #   sc (scores, 2 banks) x2 + outp x2 (1 bank ea) + po (1) + tp (1)
        main_ps = ctx.enter_context(
            tc.tile_pool(name="main_ps", bufs=1, space="PSUM"))
        att_sb = ctx.enter_context(tc.tile_pool(name="att_sb", bufs=34))
        norm_sb = ctx.enter_context(tc.tile_pool(name="norm_sb", bufs=4))
        stage = ctx.enter_context(tc.tile_pool(name="stage", bufs=4))

        # v^T -> v (natural [l, d] layout) via PE transpose
        for lc in range(B * LCB):
            tpv = main_ps.tile([128, 128], BF16, tag="tp", bufs=1, name="tpv")
            nc.tensor.transpose(
                tpv[:], vT_sb[:, lc * 128:(lc + 1) * 128], ident_b[:])
            nc.vector.tensor_copy(v_sb[:, lc, 0:D], tpv[:])

        pending = deque()

        def drain(n):
            for _ in range(min(n, len(pending))):
                pending.popleft()()

        def oproj_unit(st2, hcx, hf, ob):
            def emit():
                po = main_ps.tile([128, 512], F32, tag="po", bufs=1,
                                  name="po")
                sl = slice(st2 * 1024 + hf * 512, st2 * 1024 + (hf + 1) * 512)
                for h in range(HPC):
                    nc.tensor.matmul(po[:], wo_sb[:, h, hcx, :],
                                     outT_sb[:, h, sl],
                                     start=(h == 0), stop=(h == HPC - 1))
                if hf == 0:
                    nc.vector.tensor_copy(ob[:, 0:512], po[:])
                else:
                    nc.scalar.copy(ob[:, 512:1024], po[:])
                if hf == 1:
                    nc.sync.dma_start(outT_d[hcx, st2], ob[:])
            return emit

        def push_oproj(st2):
            for hcx in range(HC):
                ob = stage.tile([128, 1024], BF16, tag="ob", name="ob")
                for hf in range(2):
                    pending.append(oproj_unit(st2, hcx, hf, ob))

        def pv_pass_unit(b, qt, h, p, exs, outp):
            # PV for qs pair (2p, 2p+1) of this group, ones-column included
            def emit():
                for i, qs in enumerate((2 * p, 2 * p + 1)):
                    cq = qt * 8 + qs
                    for lc in range(cq + 1):
                        nc.tensor.matmul(
                            outp[i][:], exs[lc][:, qs * 128:(qs + 1) * 128],
                            v_sb[:, b * LCB + lc, :],
                            start=(lc == 0), stop=(lc == cq))
            return emit

        def norm_unit(b, qt, h, p, outp, nms):
            def emit():
                for i, qs in enumerate((2 * p, 2 * p + 1)):
                    o = outp[i]
                    rc = norm_sb.tile([128, 1], F32, tag="rc", name="rc")
                    nc.vector.reciprocal(rc[:], o[:, D:D + 1])
                    nm = norm_sb.tile([128, 128], BF16, tag="nm", name="nm")
                    nc.vector.tensor_scalar_mul(nm[:], o[:, 0:D], rc[:])
                    nms.append((qs, nm))
            return emit

        def tp_unit(b, qt, h, nms):
            def emit():
                qs, nm = nms.popleft()
                tp = main_ps.tile([128, 128], BF16, tag="tp", bufs=1,
                                  name="tp")
                nc.tensor.transpose(tp[:], nm[:], ident_b[:])
                qglob = b * S + qt * 1024 + qs * 128
                nc.vector.tensor_copy(outT_sb[:, h, qglob:qglob + 128], tp[:])
            return emit

        def push_group_tail(b, qt, h, exs):
            # PV passes + normalize + transpose units for a finished group
            nms = deque()
            for p in range(4):
                outp = [main_ps.tile([128, 129], F32, tag="outp", bufs=2,
                                     name=f"outp{p}_{i}") for i in range(2)]
                pending.append(pv_pass_unit(b, qt, h, p, exs, outp))
                pending.append(norm_unit(b, qt, h, p, outp, nms))
                pending.append(tp_unit(b, qt, h, nms))
                pending.append(tp_unit(b, qt, h, nms))

        # ---- main loop over (b, qt, h) groups ----
        for b in range(B):
            qoff = b * S
            for qt in range(QT):
                for h in range(HPC):
                    Q = qt * 1024
                    nl = qt * 8 + 8
                    exs = []
                    for lc in range(nl):
                        m = lc - Q // 128  # >=0: diagonal block index
                        w0 = max(m, 0) * 128
                        sc = main_ps.tile([128, 1024], F32, tag="sc", bufs=2,
                                          name="sc")
                        kchunk = kT_sb[:, qoff + lc * 128:
                                       qoff + (lc + 1) * 128]
                        if w0 < 512:
                            nc.tensor.matmul(
                                sc[:, w0:512], kchunk,
                                qT_sb[:, h, qoff + Q + w0:qoff + Q + 512],
                                start=True, stop=True)
                        w1 = max(w0, 512)
                        nc.tensor.matmul(
                            sc[:, w1:1024], kchunk,
                            qT_sb[:, h, qoff + Q + w1:qoff + Q + 1024],
                            start=True, stop=True)
                        ex = att_sb.tile([128, 1024], BF16, tag="ex",
                                         name="ex")
                        if w0 < 512:
                            nc.scalar.activation(
                                ex[:, w0:512], sc[:, w0:512],
                                mybir.ActivationFunctionType.Exp, scale=SCALE)
                        nc.scalar.activation(
                            ex[:, w1:1024], sc[:, w1:1024],
                            mybir.ActivationFunctionType.Exp, scale=SCALE)
                        if m >= 0:
                            nc.vector.tensor_mul(
                                ex[:, w0:w0 + 128], ex[:, w0:w0 + 128],
                                tri_sb[:])
                        exs.append(ex)
                        drain(2)
                    push_group_tail(b, qt, h, exs)
            # o-proj units chase this batch's group tails in the FIFO, so
            # every outT_sb write is emitted before the o-proj reading it
            push_oproj(2 * b)
            push_oproj(2 * b + 1)
        drain(len(pending))

    nc.compile()
    _cache["nc"] = nc
    return nc


def _bf(x):
    return np.ascontiguousarray(x).astype(ml_dtypes.bfloat16)


def _in_maps(inputs):
    hs = np.asarray(inputs["hidden_states"], np.float32).reshape(BS, HID).T
    hsT = _bf(hs)                                   # [HID, BS]
    # tile into contiguous [st2, hc, 128, 1024] blocks
    hsTt = np.ascontiguousarray(
        hsT.reshape(HC, 128, ST2, 1024).transpose(2, 0, 1, 3))
    Wq = np.asarray(inputs["Wq"], np.float32)
    Wkv = np.asarray(inputs["Wkv"], np.float32)
    Wo = np.asarray(inputs["Wo"], np.float32)

    wkT = _bf(Wkv[:D, :].T)
    wvT = _bf(Wkv[D:2 * D, :].T)
    Wq_r = Wq.reshape(H, D_NOPE + D_ROPE, HID)

    in_maps = []
    for c in range(N_CORES):
        heads = range(c * HPC, (c + 1) * HPC)
        wqT = _bf(np.concatenate([Wq_r[h, :D_NOPE, :] for h in heads], 0).T)
        woT = _bf(np.concatenate(
            [Wo[:, h * D:(h + 1) * D].T for h in heads], 0))
        in_maps.append({
            "hsTt": hsTt, "wqT": wqT, "wkT": wkT, "wvT": wvT, "woT": woT,
        })
    return in_maps


def _gather(results):
    acc = results[0]["outTt"].astype(np.float32)
    for r in results[1:]:
        acc = acc + r["outTt"].astype(np.float32)
    # [hc, st2, 128, 1024] -> outT [HID, BS] -> [B, S, HID]
    outT = acc.transpose(0, 2, 1, 3).reshape(HID, BS)
    return np.ascontiguousarray(outT.T).reshape(B, S, HID)


def run(inputs, trace=False, **kw):
    nc = _build()
    res = run_bass_kernel_spmd(nc, _in_maps(inputs), list(range(N_CORES)),
                               trace=trace, **kw)
    return _gather(res.results), res


def kernel(**inputs):
    out, _ = run(inputs)
    return out


# revision 18
# speedup vs baseline: 1.3208x; 1.3208x over previous
"""DeepSeekV2-style MLA prefill attention on 8 Trainium2 NeuronCores.

Tensor-parallel over heads: each core owns 2 of the 16 q heads (q_nope only),
replicates the single latent kv head, computes its heads' causal attention
and a partial o-proj; the host sums the 8 partial outputs.

v2 design notes (driven by the p-state clock model: PE runs 0.65/1.2 GHz
after any idle gap and only reaches 2.4 GHz after 3us of continuous work, so
the whole kernel is scheduled to keep the tensor engine hot):

  - warmup transposes at t=0 start the clock ramp during the first DMA wait
  - proj phase is split into two 4-bank PSUM half-passes per s-tile so bank
    drains overlap the next pass's fills (no st2-boundary stall); v is
    computed directly in natural [l, d] layout (hs slice as the stationary
    operand), eliminating the 32 PE transposes of v
  - attention q-tiles are 1024 wide: one exp() activation per 2-bank PSUM
    scores tile halves the scalar engine's per-call overhead
  - the scalar engine does ONLY exp; every PSUM->SBUF drain runs on the
    Pool (gpsimd) or DVE (vector) engine
  - the scores loop is exp-gated (~1us exp vs ~430ns of matmul per l-chunk),
    so a work queue interleaves the PREVIOUS group's PV passes / normalizes
    and pending o-proj column blocks into the gaps, in program order
"""

import numpy as np
import ml_dtypes
from collections import deque
from contextlib import ExitStack

import concourse.bass as bass
import concourse.mybir as mybir
import concourse.tile as tile
from concourse import bacc
from concourse.bass_utils import run_bass_kernel_spmd
from concourse.masks import make_identity

B, S, HID = 2, 2048, 2048
H, D_NOPE, D_ROPE = 16, 128, 64
D = D_NOPE
N_CORES = 8
HPC = H // N_CORES          # heads per core
BS = B * S
SCALE = float(D_NOPE) ** -0.5

F32 = mybir.dt.float32
BF16 = mybir.dt.bfloat16

HC = HID // 128             # 16 hid chunks
ST2 = BS // 1024            # 4 wide s tiles
LCB = S // 128              # 16 l chunks per batch
QT = S // 1024              # 2 q tiles (1024 wide) per batch

_cache = {}


def _build():
    if "nc" in _cache:
        return _cache["nc"]

    nc = bacc.Bacc("TRN2", target_bir_lowering=False, debug=False,
                   num_devices=N_CORES)
    # hsT tiled: [st2, hid_chunk, 128, 1024] contiguous blocks
    hsT_d = nc.dram_tensor("hsTt", [ST2, HC, 128, 1024], BF16,
                           kind="ExternalInput").ap()
    wqT_d = nc.dram_tensor("wqT", [HID, HPC * D], BF16, kind="ExternalInput").ap()
    wkT_d = nc.dram_tensor("wkT", [HID, D], BF16, kind="ExternalInput").ap()
    wvT_d = nc.dram_tensor("wvT", [HID, D], BF16, kind="ExternalInput").ap()
    woT_d = nc.dram_tensor("woT", [HPC * D, HID], BF16, kind="ExternalInput").ap()
    # output tiled: [hid_chunk, st2, 128, 1024]
    outT_d = nc.dram_tensor("outTt", [HC, ST2, 128, 1024], BF16,
                            kind="ExternalOutput").ap()

    with ExitStack() as ctx:
        tc = ctx.enter_context(tile.TileContext(nc))
        persist = ctx.enter_context(tc.tile_pool(name="persist", bufs=1))

        wq_sb = persist.tile([128, HC, HPC * D], BF16, tag="wq_sb")
        wk_sb = persist.tile([128, HC, D], BF16, tag="wk_sb")
        wv_sb = persist.tile([128, HC, D], BF16, tag="wv_sb")
        wo_sb = persist.tile([128, HPC, HC, 128], BF16, tag="wo_sb")
        qT_sb = persist.tile([128, HPC, BS], BF16, tag="qT_sb")
        kT_sb = persist.tile([128, BS], BF16, tag="kT_sb")
        vT_sb = persist.tile([128, BS], BF16, tag="vT_sb")
        v_sb = persist.tile([128, B * LCB, D + 1], BF16, tag="v_sb")
        tri_f = persist.tile([128, 128], F32, tag="tri_f")
        tri_sb = persist.tile([128, 128], BF16, tag="tri_sb")
        ident_b = persist.tile([128, 128], BF16, tag="ident_b")
        outT_sb = persist.tile([128, HPC, BS], BF16, tag="outT_sb")

        # ---- constants ----
        wq_src = wqT_d.rearrange("(c p) m -> p c m", p=128)
        wk_src = wkT_d.rearrange("(c p) m -> p c m", p=128)
        wv_src = wvT_d.rearrange("(c p) m -> p c m", p=128)
        make_identity(nc, ident_b[:])
        # tri[x, y] = 1 where x <= y else 0 (diag-block causal mask)
        nc.gpsimd.memset(tri_f[:], 1.0)
        nc.gpsimd.affine_select(
            out=tri_f[:], in_=tri_f[:],
            compare_op=mybir.AluOpType.is_ge,
            fill=0.0, base=0,
            pattern=[[1, 128]], channel_multiplier=-1)
        nc.vector.tensor_copy(tri_sb[:], tri_f[:])
        nc.vector.memset(v_sb[:, :, D:D + 1], 1.0)

        # ---- phase 1: projections ----
        # per st2: pass A (hf=0 q/k halves + all 8 natural-layout v tiles),
        # pass B (hf=1 q/k halves) while pass A's banks drain.
        with tc.tile_pool(name="ps_proj", bufs=1, space="PSUM") as ps_proj, \
             tc.tile_pool(name="hs_pool", bufs=18) as hs_pool:
            # warmup: keep PE busy during the initial DMA wait so the clock
            # ramp starts immediately (values are discarded: the first v
            # matmul below starts a fresh accumulation group).
            pv_warm = ps_proj.tile([128, 512], F32, tag="pv", bufs=2,
                                   name="pv_warm")
            for w in range(8):
                nc.tensor.transpose(pv_warm[:, 0:128], tri_f[:], tri_f[:])

            # st2=0 tiles + weights issued upfront; later st2 tiles are
            # issued inside the previous st2's hf=1 sweep (right as each
            # buffer frees) so the DMA engine prefetches a full tile ahead
            nxt = []
            for hcx in range(HC):
                hst = hs_pool.tile([128, 1024], BF16, tag="hst")
                nc.sync.dma_start(hst[:], hsT_d[0, hcx])
                nxt.append(hst)
                nc.sync.dma_start(wq_sb[:, hcx, :], wq_src[:, hcx, :])
                nc.sync.dma_start(wk_sb[:, hcx, :], wk_src[:, hcx, :])
                nc.sync.dma_start(wv_sb[:, hcx, :], wv_src[:, hcx, :])
                if hcx == HC - 1:
                    nc.sync.dma_start(
                        wo_sb[:],
                        woT_d.rearrange("(h p) (c m) -> p h c m",
                                        p=128, m=128))
            for st2 in range(ST2):
                hsts = nxt
                nxt = []
                for hf in range(2):
                    pq = [ps_proj.tile([128, 512], F32, tag=f"pq{h}", bufs=2,
                                       name=f"pq{h}_{hf}") for h in range(HPC)]
                    pk = ps_proj.tile([128, 512], F32, tag="pk", bufs=2,
                                      name=f"pk_{hf}")
                    pv = ps_proj.tile([128, 512], F32, tag="pv", bufs=2,
                                      name=f"pv_{hf}")
                    for hcx in range(HC):
                        hst = hsts[hcx]
                        hr = hst[:, hf * 512:(hf + 1) * 512]
                        first, last = hcx == 0, hcx == HC - 1
                        for h in range(HPC):
                            nc.tensor.matmul(
                                pq[h][:], wq_sb[:, hcx, h * D:(h + 1) * D],
                                hr, start=first, stop=last)
                        nc.tensor.matmul(pk[:], wk_sb[:, hcx, :], hr,
                                         start=first, stop=last)
                        nc.tensor.matmul(pv[:], wv_sb[:, hcx, :], hr,
                                         start=first, stop=last)
                        if hf == 1 and st2 < ST2 - 1:
                            hst2 = hs_pool.tile([128, 1024], BF16, tag="hst")
                            nc.sync.dma_start(hst2[:], hsT_d[st2 + 1, hcx])
                            nxt.append(hst2)
                    sl = slice(st2 * 1024 + hf * 512,
                               st2 * 1024 + (hf + 1) * 512)
                    for h in range(HPC):
                        if h == 0:
                            nc.scalar.copy(qT_sb[:, h, sl], pq[h][:])
                        else:
                            nc.vector.tensor_copy(qT_sb[:, h, sl], pq[h][:])
                    nc.scalar.copy(kT_sb[:, sl], pk[:])
                    nc.vector.tensor_copy(vT_sb[:, sl], pv[:])

        # ---- phase 2: attention + o-proj, one shared 8-bank PSUM pool ----
        #   sc (scores, 2 banks) x2 + outp x2 (1 bank ea) + po (1) + tp (1)
        main_ps = ctx.enter_context(
            tc.tile_pool(name="main_ps", bufs=1, space="PSUM"))
        att_sb = ctx.enter_context(tc.tile_pool(name="att_sb", bufs=34))
        norm_sb = ctx.enter_context(tc.tile_pool(name="norm_sb", bufs=4))
        stage = ctx.enter_context(tc.tile_pool(name="stage", bufs=4))

        pending = deque()

        def drain(n):
            for _ in range(min(n, len(pending))):
                pending.popleft()()

        def vtp_unit(lc):
            # v^T -> v (natural [l, d] layout) via PE transpose
            def emit():
                tpv = main_ps.tile([128, 128], BF16, tag="tp", bufs=1,
                                   name="tpv")
                nc.tensor.transpose(
                    tpv[:], vT_sb[:, lc * 128:(lc + 1) * 128], ident_b[:])
                nc.vector.tensor_copy(v_sb[:, lc, 0:D], tpv[:])
            return emit

        for lc in range(B * LCB):
            pending.append(vtp_unit(lc))

        def oproj_unit(st2, hcx, hf, ob):
            def emit():
                po = main_ps.tile([128, 512], F32, tag="po", bufs=2,
                                  name="po")
                sl = slice(st2 * 1024 + hf * 512, st2 * 1024 + (hf + 1) * 512)
                for h in range(HPC):
                    nc.tensor.matmul(po[:], wo_sb[:, h, hcx, :],
                                     outT_sb[:, h, sl],
                                     start=(h == 0), stop=(h == HPC - 1))
                if hf == 0:
                    nc.vector.tensor_copy(ob[:, 0:512], po[:])
                else:
                    nc.scalar.copy(ob[:, 512:1024], po[:])
                if hf == 1:
                    nc.sync.dma_start(outT_d[hcx, st2], ob[:])
            return emit

        def push_oproj(st2):
            for hcx in range(HC):
                ob = stage.tile([128, 1024], BF16, tag="ob", name="ob")
                for hf in range(2):
                    pending.append(oproj_unit(st2, hcx, hf, ob))

        def pv_pass_unit(b, qt, h, p, exs, outp):
            # PV for qs pair (2p, 2p+1) of this group, ones-column included
            def emit():
                for i, qs in enumerate((2 * p, 2 * p + 1)):
                    cq = qt * 8 + qs
                    for lc in range(cq + 1):
                        nc.tensor.matmul(
                            outp[i][:], exs[lc][:, qs * 128:(qs + 1) * 128],
                            v_sb[:, b * LCB + lc, :],
                            start=(lc == 0), stop=(lc == cq))
            return emit

        def norm_unit(b, qt, h, p, outp, nms):
            def emit():
                for i, qs in enumerate((2 * p, 2 * p + 1)):
                    o = outp[i]
                    rc = norm_sb.tile([128, 1], F32, tag="rc", name="rc")
                    nc.vector.reciprocal(rc[:], o[:, D:D + 1])
                    nm = norm_sb.tile([128, 128], BF16, tag="nm", name="nm")
                    nc.vector.tensor_scalar_mul(nm[:], o[:, 0:D], rc[:])
                    nms.append((qs, nm))
            return emit

        def tp_unit(b, qt, h, nms):
            def emit():
                qs, nm = nms.popleft()
                tp = main_ps.tile([128, 128], BF16, tag="tp", bufs=1,
                                  name="tp")
                nc.tensor.transpose(tp[:], nm[:], ident_b[:])
                qglob = b * S + qt * 1024 + qs * 128
                nc.vector.tensor_copy(outT_sb[:, h, qglob:qglob + 128], tp[:])
            return emit

        def push_group_tail(b, qt, h, exs):
            # PV passes + normalize + transpose units for a finished group
            nms = deque()
            for p in range(4):
                outp = [main_ps.tile([128, 129], F32, tag="outp", bufs=2,
                                     name=f"outp{p}_{i}") for i in range(2)]
                pending.append(pv_pass_unit(b, qt, h, p, exs, outp))
                pending.append(norm_unit(b, qt, h, p, outp, nms))
                pending.append(tp_unit(b, qt, h, nms))
                pending.append(tp_unit(b, qt, h, nms))

        # ---- main loop over (b, qt, h) groups ----
        for b in range(B):
            qoff = b * S
            for qt in range(QT):
                for h in range(HPC):
                    Q = qt * 1024
                    nl = qt * 8 + 8
                    exs = []
                    for lc in range(nl):
                        m = lc - Q // 128  # >=0: diagonal block index
                        w0 = max(m, 0) * 128
                        kchunk = kT_sb[:, qoff + lc * 128:
                                       qoff + (lc + 1) * 128]
                        ex = att_sb.tile([128, 1024], BF16, tag="ex",
                                         name="ex")
                        if w0 < 512:
                            scA = main_ps.tile([128, 512], F32, tag="sc",
                                               bufs=3, name="scA")
                            nc.tensor.matmul(
                                scA[:, w0:512], kchunk,
                                qT_sb[:, h, qoff + Q + w0:qoff + Q + 512],
                                start=True, stop=True)
                            nc.scalar.activation(
                                ex[:, w0:512], scA[:, w0:512],
                                mybir.ActivationFunctionType.Exp, scale=SCALE)
                            drain(1)
                        w1 = max(w0, 512) - 512
                        scB = main_ps.tile([128, 512], F32, tag="sc",
                                           bufs=3, name="scB")
                        nc.tensor.matmul(
                            scB[:, w1:512], kchunk,
                            qT_sb[:, h, qoff + Q + 512 + w1:qoff + Q + 1024],
                            start=True, stop=True)
                        nc.scalar.activation(
                            ex[:, 512 + w1:1024], scB[:, w1:512],
                            mybir.ActivationFunctionType.Exp, scale=SCALE)
                        if m >= 0:
                            nc.vector.tensor_mul(
                                ex[:, w0:w0 + 128], ex[:, w0:w0 + 128],
                                tri_sb[:])
                        exs.append(ex)
                        drain(1)
                    push_group_tail(b, qt, h, exs)
            # o-proj units chase this batch's group tails in the FIFO, so
            # every outT_sb write is emitted before the o-proj reading it
            push_oproj(2 * b)
            push_oproj(2 * b + 1)
        drain(len(pending))

    nc.compile()
    _cache["nc"] = nc
    return nc


def _bf(x):
    return np.ascontiguousarray(x).astype(ml_dtypes.bfloat16)


def _in_maps(inputs):
    hs = np.asarray(inputs["hidden_states"], np.float32).reshape(BS, HID).T
    hsT = _bf(hs)                                   # [HID, BS]
    # tile into contiguous [st2, hc, 128, 1024] blocks
    hsTt = np.ascontiguousarray(
        hsT.reshape(HC, 128, ST2, 1024).transpose(2, 0, 1, 3))
    Wq = np.asarray(inputs["Wq"], np.float32)
    Wkv = np.asarray(inputs["Wkv"], np.float32)
    Wo = np.asarray(inputs["Wo"], np.float32)

    wkT = _bf(Wkv[:D, :].T)
    wvT = _bf(Wkv[D:2 * D, :].T)
    Wq_r = Wq.reshape(H, D_NOPE + D_ROPE, HID)

    in_maps = []
    for c in range(N_CORES):
        heads = range(c * HPC, (c + 1) * HPC)
        wqT = _bf(np.concatenate([Wq_r[h, :D_NOPE, :] for h in heads], 0).T)
        woT = _bf(np.concatenate(
            [Wo[:, h * D:(h + 1) * D].T for h in heads], 0))
        in_maps.append({
            "hsTt": hsTt, "wqT": wqT, "wkT": wkT, "wvT": wvT, "woT": woT,
        })
    return in_maps


def _gather(results):
    acc = results[0]["outTt"].astype(np.float32)
    for r in results[1:]:
        acc = acc + r["outTt"].astype(np.float32)
    # [hc, st2, 128, 1024] -> outT [HID, BS] -> [B, S, HID]
    outT = acc.transpose(0, 2, 1, 3).reshape(HID, BS)
    return np.ascontiguousarray(outT.T).reshape(B, S, HID)


def run(inputs, trace=False, **kw):
    nc = _build()
    res = run_bass_kernel_spmd(nc, _in_maps(inputs), list(range(N_CORES)),
                               trace=trace, **kw)
    return _gather(res.results), res


def kernel(**inputs):
    out, _ = run(inputs)
    return out


# revision 21
# speedup vs baseline: 1.3311x; 1.0078x over previous
"""DeepSeekV2-style MLA prefill attention on 8 Trainium2 NeuronCores.

Tensor-parallel over heads: each core owns 2 of the 16 q heads (q_nope only),
replicates the single latent kv head, computes its heads' causal attention
and a partial o-proj; the host sums the 8 partial outputs.

v2 design notes (driven by the p-state clock model: PE runs 0.65/1.2 GHz
after any idle gap and only reaches 2.4 GHz after 3us of continuous work, so
the whole kernel is scheduled to keep the tensor engine hot):

  - warmup transposes at t=0 start the clock ramp during the first DMA wait
  - proj phase is split into two 4-bank PSUM half-passes per s-tile so bank
    drains overlap the next pass's fills (no st2-boundary stall); v is
    computed directly in natural [l, d] layout (hs slice as the stationary
    operand), eliminating the 32 PE transposes of v
  - attention q-tiles are 1024 wide: one exp() activation per 2-bank PSUM
    scores tile halves the scalar engine's per-call overhead
  - the scalar engine does ONLY exp; every PSUM->SBUF drain runs on the
    Pool (gpsimd) or DVE (vector) engine
  - the scores loop is exp-gated (~1us exp vs ~430ns of matmul per l-chunk),
    so a work queue interleaves the PREVIOUS group's PV passes / normalizes
    and pending o-proj column blocks into the gaps, in program order
"""

import numpy as np
import ml_dtypes
from collections import deque
from contextlib import ExitStack

import concourse.bass as bass
import concourse.mybir as mybir
import concourse.tile as tile
from concourse import bacc
from concourse.bass_utils import run_bass_kernel_spmd
from concourse.masks import make_identity

B, S, HID = 2, 2048, 2048
H, D_NOPE, D_ROPE = 16, 128, 64
D = D_NOPE
N_CORES = 8
HPC = H // N_CORES          # heads per core
BS = B * S
SCALE = float(D_NOPE) ** -0.5

F32 = mybir.dt.float32
BF16 = mybir.dt.bfloat16

HC = HID // 128             # 16 hid chunks
ST2 = BS // 1024            # 4 wide s tiles
LCB = S // 128              # 16 l chunks per batch
QT = S // 1024              # 2 q tiles (1024 wide) per batch

_cache = {}


def _build():
    if "nc" in _cache:
        return _cache["nc"]

    nc = bacc.Bacc("TRN2", target_bir_lowering=False, debug=False,
                   num_devices=N_CORES)
    # hsT tiled: [st2, hid_chunk, 128, 1024] contiguous blocks
    hsT_d = nc.dram_tensor("hsTt", [ST2, HC, 128, 1024], BF16,
                           kind="ExternalInput").ap()
    wqT_d = nc.dram_tensor("wqT", [HID, HPC * D], BF16, kind="ExternalInput").ap()
    wkT_d = nc.dram_tensor("wkT", [HID, D], BF16, kind="ExternalInput").ap()
    wvT_d = nc.dram_tensor("wvT", [HID, D], BF16, kind="ExternalInput").ap()
    woT_d = nc.dram_tensor("woT", [HPC * D, HID], BF16, kind="ExternalInput").ap()
    # output tiled: [hid_chunk, st2, 128, 1024]
    outT_d = nc.dram_tensor("outTt", [HC, ST2, 128, 1024], BF16,
                            kind="ExternalOutput").ap()

    with ExitStack() as ctx:
        tc = ctx.enter_context(tile.TileContext(nc))
        persist = ctx.enter_context(tc.tile_pool(name="persist", bufs=1))

        wq_sb = persist.tile([128, HC, HPC * D], BF16, tag="wq_sb")
        wk_sb = persist.tile([128, HC, D], BF16, tag="wk_sb")
        wv_sb = persist.tile([128, HC, D], BF16, tag="wv_sb")
        wo_sb = persist.tile([128, HPC, HC, 128], BF16, tag="wo_sb")
        qT_sb = persist.tile([128, HPC, BS], BF16, tag="qT_sb")
        kT_sb = persist.tile([128, BS], BF16, tag="kT_sb")
        vT_sb = persist.tile([128, BS], BF16, tag="vT_sb")
        v_sb = persist.tile([128, B * LCB, D + 1], BF16, tag="v_sb")
        tri_f = persist.tile([128, 128], F32, tag="tri_f")
        tri_sb = persist.tile([128, 128], BF16, tag="tri_sb")
        ident_b = persist.tile([128, 128], BF16, tag="ident_b")
        outT_sb = persist.tile([128, HPC, BS], BF16, tag="outT_sb")

        # ---- constants ----
        wq_src = wqT_d.rearrange("(c p) m -> p c m", p=128)
        wk_src = wkT_d.rearrange("(c p) m -> p c m", p=128)
        wv_src = wvT_d.rearrange("(c p) m -> p c m", p=128)
        make_identity(nc, ident_b[:])
        # tri[x, y] = 1 where x <= y else 0 (diag-block causal mask)
        nc.gpsimd.memset(tri_f[:], 1.0)
        nc.gpsimd.affine_select(
            out=tri_f[:], in_=tri_f[:],
            compare_op=mybir.AluOpType.is_ge,
            fill=0.0, base=0,
            pattern=[[1, 128]], channel_multiplier=-1)
        nc.vector.tensor_copy(tri_sb[:], tri_f[:])
        nc.vector.memset(v_sb[:, :, D:D + 1], 1.0)

        # ---- phase 1: projections ----
        # per st2: pass A (hf=0 q/k halves + all 8 natural-layout v tiles),
        # pass B (hf=1 q/k halves) while pass A's banks drain.
        with tc.tile_pool(name="ps_proj", bufs=1, space="PSUM") as ps_proj, \
             tc.tile_pool(name="hs_pool", bufs=18) as hs_pool:
            # warmup: keep PE busy during the initial DMA wait so the clock
            # ramp starts immediately (values are discarded: the first v
            # matmul below starts a fresh accumulation group).
            pv_warm = ps_proj.tile([128, 512], F32, tag="pv", bufs=2,
                                   name="pv_warm")
            for w in range(20):
                nc.tensor.transpose(pv_warm[:, 0:128], tri_f[:], tri_f[:])

            # st2=0 tiles + weights issued upfront; later st2 tiles are
            # issued inside the previous st2's hf=1 sweep (right as each
            # buffer frees) so the DMA engine prefetches a full tile ahead
            nxt = []
            for hcx in range(HC):
                hst = hs_pool.tile([128, 1024], BF16, tag="hst")
                nc.sync.dma_start(hst[:], hsT_d[0, hcx])
                nxt.append(hst)
                nc.sync.dma_start(wq_sb[:, hcx, :], wq_src[:, hcx, :])
                nc.sync.dma_start(wk_sb[:, hcx, :], wk_src[:, hcx, :])
                nc.sync.dma_start(wv_sb[:, hcx, :], wv_src[:, hcx, :])
                if hcx == HC - 1:
                    nc.sync.dma_start(
                        wo_sb[:],
                        woT_d.rearrange("(h p) (c m) -> p h c m",
                                        p=128, m=128))
            for st2 in range(ST2):
                hsts = nxt
                nxt = []
                for hf in range(2):
                    pq = [ps_proj.tile([128, 512], F32, tag=f"pq{h}", bufs=2,
                                       name=f"pq{h}_{hf}") for h in range(HPC)]
                    pk = ps_proj.tile([128, 512], F32, tag="pk", bufs=2,
                                      name=f"pk_{hf}")
                    pv = ps_proj.tile([128, 512], F32, tag="pv", bufs=2,
                                      name=f"pv_{hf}")
                    for hcx in range(HC):
                        hst = hsts[hcx]
                        hr = hst[:, hf * 512:(hf + 1) * 512]
                        first, last = hcx == 0, hcx == HC - 1
                        for h in range(HPC):
                            nc.tensor.matmul(
                                pq[h][:], wq_sb[:, hcx, h * D:(h + 1) * D],
                                hr, start=first, stop=last)
                        nc.tensor.matmul(pk[:], wk_sb[:, hcx, :], hr,
                                         start=first, stop=last)
                        nc.tensor.matmul(pv[:], wv_sb[:, hcx, :], hr,
                                         start=first, stop=last)
                        if hf == 1 and st2 < ST2 - 1:
                            hst2 = hs_pool.tile([128, 1024], BF16, tag="hst")
                            nc.sync.dma_start(hst2[:], hsT_d[st2 + 1, hcx])
                            nxt.append(hst2)
                    sl = slice(st2 * 1024 + hf * 512,
                               st2 * 1024 + (hf + 1) * 512)
                    for h in range(HPC):
                        if h == 0:
                            nc.scalar.copy(qT_sb[:, h, sl], pq[h][:])
                        else:
                            nc.vector.tensor_copy(qT_sb[:, h, sl], pq[h][:])
                    nc.scalar.copy(kT_sb[:, sl], pk[:])
                    nc.vector.tensor_copy(vT_sb[:, sl], pv[:])

        # ---- phase 2: attention + o-proj, one shared 8-bank PSUM pool ----
        #   sc (scores, 2 banks) x2 + outp x2 (1 bank ea) + po (1) + tp (1)
        main_ps = ctx.enter_context(
            tc.tile_pool(name="main_ps", bufs=1, space="PSUM"))
        att_sb = ctx.enter_context(tc.tile_pool(name="att_sb", bufs=34))
        norm_sb = ctx.enter_context(tc.tile_pool(name="norm_sb", bufs=4))
        stage = ctx.enter_context(tc.tile_pool(name="stage", bufs=8))

        pending = deque()

        def drain(n):
            for _ in range(min(n, len(pending))):
                pending.popleft()()

        def vtp_unit(lc):
            # v^T -> v (natural [l, d] layout) via PE transpose
            def emit():
                tpv = main_ps.tile([128, 128], BF16, tag="tp", bufs=1,
                                   name="tpv")
                nc.tensor.transpose(
                    tpv[:], vT_sb[:, lc * 128:(lc + 1) * 128], ident_b[:])
                nc.vector.tensor_copy(v_sb[:, lc, 0:D], tpv[:])
            return emit

        for lc in range(B * LCB):
            pending.append(vtp_unit(lc))

        def oproj_unit(st2, hcx, hf, ob):
            def emit():
                po = main_ps.tile([128, 512], F32, tag="po", bufs=2,
                                  name="po")
                sl = slice(st2 * 1024 + hf * 512, st2 * 1024 + (hf + 1) * 512)
                for h in range(HPC):
                    nc.tensor.matmul(po[:], wo_sb[:, h, hcx, :],
                                     outT_sb[:, h, sl],
                                     start=(h == 0), stop=(h == HPC - 1))
                if hf == 0:
                    nc.vector.tensor_copy(ob[:, 0:512], po[:])
                else:
                    nc.scalar.copy(ob[:, 512:1024], po[:])
                if hf == 1:
                    nc.sync.dma_start(outT_d[hcx, st2], ob[:])
            return emit

        def push_oproj(st2):
            for hcx in range(HC):
                ob = stage.tile([128, 1024], BF16, tag="ob", name="ob")
                for hf in range(2):
                    pending.append(oproj_unit(st2, hcx, hf, ob))

        def pv_pass_unit(b, qt, h, p, exs, outp):
            # PV for qs pair (2p, 2p+1) of this group, ones-column included
            def emit():
                for i, qs in enumerate((2 * p, 2 * p + 1)):
                    cq = qt * 8 + qs
                    for lc in range(cq + 1):
                        nc.tensor.matmul(
                            outp[i][:], exs[lc][:, qs * 128:(qs + 1) * 128],
                            v_sb[:, b * LCB + lc, :],
                            start=(lc == 0), stop=(lc == cq))
            return emit

        def norm_unit(b, qt, h, p, outp, nms):
            def emit():
                for i, qs in enumerate((2 * p, 2 * p + 1)):
                    o = outp[i]
                    rc = norm_sb.tile([128, 1], F32, tag="rc", name="rc")
                    nc.vector.reciprocal(rc[:], o[:, D:D + 1])
                    nm = norm_sb.tile([128, 128], BF16, tag="nm", name="nm")
                    nc.vector.tensor_scalar_mul(nm[:], o[:, 0:D], rc[:])
                    nms.append((qs, nm))
            return emit

        def tp_unit(b, qt, h, nms):
            def emit():
                qs, nm = nms.popleft()
                tp = main_ps.tile([128, 128], BF16, tag="tp", bufs=1,
                                  name="tp")
                nc.tensor.transpose(tp[:], nm[:], ident_b[:])
                qglob = b * S + qt * 1024 + qs * 128
                nc.vector.tensor_copy(outT_sb[:, h, qglob:qglob + 128], tp[:])
            return emit

        def push_group_tail(b, qt, h, exs):
            # PV passes + normalize + transpose units for a finished group
            nms = deque()
            for p in range(4):
                outp = [main_ps.tile([128, 129], F32, tag="outp", bufs=2,
                                     name=f"outp{p}_{i}") for i in range(2)]
                pending.append(pv_pass_unit(b, qt, h, p, exs, outp))
                pending.append(norm_unit(b, qt, h, p, outp, nms))
                pending.append(tp_unit(b, qt, h, nms))
                pending.append(tp_unit(b, qt, h, nms))

        # ---- main loop over (b, qt, h) groups ----
        for b in range(B):
            qoff = b * S
            for qt in range(QT):
                for h in range(HPC):
                    Q = qt * 1024
                    nl = qt * 8 + 8
                    exs = []
                    for lc in range(nl):
                        m = lc - Q // 128  # >=0: diagonal block index
                        w0 = max(m, 0) * 128
                        kchunk = kT_sb[:, qoff + lc * 128:
                                       qoff + (lc + 1) * 128]
                        ex = att_sb.tile([128, 1024], BF16, tag="ex",
                                         name="ex")
                        if w0 < 512:
                            scA = main_ps.tile([128, 512], F32, tag="sc",
                                               bufs=3, name="scA")
                            nc.tensor.matmul(
                                scA[:, w0:512], kchunk,
                                qT_sb[:, h, qoff + Q + w0:qoff + Q + 512],
                                start=True, stop=True)
                            nc.scalar.activation(
                                ex[:, w0:512], scA[:, w0:512],
                                mybir.ActivationFunctionType.Exp, scale=SCALE)
                            drain(1)
                        w1 = max(w0, 512) - 512
                        scB = main_ps.tile([128, 512], F32, tag="sc",
                                           bufs=3, name="scB")
                        nc.tensor.matmul(
                            scB[:, w1:512], kchunk,
                            qT_sb[:, h, qoff + Q + 512 + w1:qoff + Q + 1024],
                            start=True, stop=True)
                        nc.scalar.activation(
                            ex[:, 512 + w1:1024], scB[:, w1:512],
                            mybir.ActivationFunctionType.Exp, scale=SCALE)
                        if m >= 0:
                            nc.vector.tensor_mul(
                                ex[:, w0:w0 + 128], ex[:, w0:w0 + 128],
                                tri_sb[:])
                        exs.append(ex)
                        drain(1)
                    push_group_tail(b, qt, h, exs)
                # o-proj units chase this s-range's group tails in the FIFO,
                # so every outT_sb write is emitted before the o-proj
                # reading it
                push_oproj(2 * b + qt)
        drain(len(pending))

    nc.compile()
    _cache["nc"] = nc
    return nc


def _bf(x):
    return np.ascontiguousarray(x).astype(ml_dtypes.bfloat16)


def _in_maps(inputs):
    hs = np.asarray(inputs["hidden_states"], np.float32).reshape(BS, HID).T
    hsT = _bf(hs)                                   # [HID, BS]
    # tile into contiguous [st2, hc, 128, 1024] blocks
    hsTt = np.ascontiguousarray(
        hsT.reshape(HC, 128, ST2, 1024).transpose(2, 0, 1, 3))
    Wq = np.asarray(inputs["Wq"], np.float32)
    Wkv = np.asarray(inputs["Wkv"], np.float32)
    Wo = np.asarray(inputs["Wo"], np.float32)

    wkT = _bf(Wkv[:D, :].T)
    wvT = _bf(Wkv[D:2 * D, :].T)
    Wq_r = Wq.reshape(H, D_NOPE + D_ROPE, HID)

    in_maps = []
    for c in range(N_CORES):
        heads = range(c * HPC, (c + 1) * HPC)
        wqT = _bf(np.concatenate([Wq_r[h, :D_NOPE, :] for h in heads], 0).T)
        woT = _bf(np.concatenate(
            [Wo[:, h * D:(h + 1) * D].T for h in heads], 0))
        in_maps.append({
            "hsTt": hsTt, "wqT": wqT, "wkT": wkT, "wvT": wvT, "woT": woT,
        })
    return in_maps


def _gather(results):
    acc = results[0]["outTt"].astype(np.float32)
    for r in results[1:]:
        acc = acc + r["outTt"].astype(np.float32)
    # [hc, st2, 128, 1024] -> outT [HID, BS] -> [B, S, HID]
    outT = acc.transpose(0, 2, 1, 3).reshape(HID, BS)
    return np.ascontiguousarray(outT.T).reshape(B, S, HID)


def run(inputs, trace=False, **kw):
    nc = _build()
    res = run_bass_kernel_spmd(nc, _in_maps(inputs), list(range(N_CORES)),
                               trace=trace, **kw)
    return _gather(res.results), res


def kernel(**inputs):
    out, _ = run(inputs)
    return out
